# revision 4
# baseline (speedup 1.0000x reference)
# Block-local matmul kernel for Trainium2 (8 NeuronCores, SPMD).
#
# Problem: out[b, i*64+r, j*64+o] = sum_c x[b, i*64+r, j*64+c] * W[i*64+c, j*64+o]
# with B=4, M=K=N=4096, 64x64 blocks. Embarrassingly parallel over (i, j).
#
# Sharding: block-row axis i across the 8 cores. Core p gets rows
# [512p, 512p+512) of x/out and rows [512p, 512p+512) of weight. No
# collectives; outputs are concatenated on the host.
#
# Per-core plan (memory-bound; HBM roofline ~200us for ~68MB):
#   - x loaded as 16 strips [128=(2 batches x 64 rows), 4096] with an
#     fp32->bf16 cast in the SWDGE DMA (SWDGE sprays 3D APs across all 16
#     SDMA engines; measured ~370GB/s in isolation).
#   - Stores are split into two 2D [64, 4096] DMAs on the two HWDGE rings
#     (nc.sync + nc.scalar). HWDGE assigns descriptors by the OUTER AP dim:
#     a 3D [2, 64, 4096] AP lands entirely on 2 SDMA engines (~54GB/s);
#     2D APs spray correctly.
#   - PE transposes each [128,128] bf16 tile into PSUM, giving A^T blocks
#     with partition = c (contraction), the matmul stationary.
#   - weight is cast to bf16 on the host; wa (natural strips) + wb
#     (partition-half swap) give W at both 64-partition alignments so rhs
#     always matches lhsT's base partition ((j%2)*64).
#   - Matmul: lhsT = A^T[c=64, (b2,r)=128] (stationary), rhs = W[c=64,o=64],
#     fp32 PSUM with NATURAL output layout -> contiguous stores.
#   - PSUM rule learned on HW: one matmul group per bank; readers may only
#     touch bytes the group wrote. Each matmul gets its own bank; a strided
#     copy gathers 3 banks, reading only the written 64-col slices.

import numpy as np

B = 4
M = K = N = 4096
NCORES = 8
RPC = M // NCORES  # 512 rows per core
NI = RPC // 64     # 8 i-blocks per core
NJ = N // 64       # 64 j-blocks

_NC_CACHE = None


def _build_nc():
    import concourse.tile as tile
    from concourse import bacc, masks, mybir

    f32 = mybir.dt.float32
    bf16 = mybir.dt.bfloat16

    nc = bacc.Bacc("TRN2", target_bir_lowering=False, debug=False,
                   num_devices=NCORES)
    x_d = nc.dram_tensor("x_shard", [B, RPC, K], bf16, kind="ExternalInput")
    w_d = nc.dram_tensor("w_shard", [RPC, N], bf16, kind="ExternalInput")
    o_d = nc.dram_tensor("out_shard", [B, RPC, N], bf16,
                         kind="ExternalOutput")

    with tile.TileContext(nc) as tc:
        with (
            tc.tile_pool(name="const", bufs=1) as constp,
            tc.tile_pool(name="wa", bufs=1) as wap,
            tc.tile_pool(name="wb", bufs=1) as wbp,
            tc.tile_pool(name="xs", bufs=3) as xp,
            tc.tile_pool(name="at", bufs=2) as atp,
            tc.tile_pool(name="ob", bufs=3) as obp,
            tc.tile_pool(name="psT", bufs=2, space="PSUM") as psTp,
            tc.tile_pool(name="psO", bufs=2, space="PSUM") as psOp,
        ):
            ident = constp.tile([128, 128], bf16)
            masks.make_identity(nc, ident[:])

            # wa[p, g, :] = weight row 128g + p. W(i) sits at partition base
            # (i%2)*64, strip g = i//2.
            wa = wap.tile([128, 4, N], bf16)
            nc.sync.dma_start(
                wa[:], w_d.ap().rearrange("(g p) n -> p g n", p=128))
            # wb = partition-half swap: odd i at base 0 (g=(i-1)//2), even i
            # at base 64 (g=i//2).
            wb = wbp.tile([128, 4, N], bf16)
            w_tpgn = w_d.ap().rearrange("(g t p) n -> t p g n", g=4, t=2, p=64)
            nc.sync.dma_start(wb[0:64, :, :], w_tpgn[1])
            nc.sync.dma_start(wb[64:128, :, :], w_tpgn[0])

            for u in range(2):        # batch pair (b in {2u, 2u+1})
                for i in range(NI):   # i-block within core
                    xs = xp.tile([128, K], bf16, tag="xs")
                    src = x_d.ap()[2 * u:2 * u + 2, 64 * i:64 * i + 64, :]
                    nc.gpsimd.dma_start(xs[:], src)

                    # Transpose all 32 [128,128] tiles; A^T(i, j) lands at
                    # partitions (j%2)*64 + c, free = (b2, r).
                    atb = atp.tile([128, 4096], bf16, tag="at")
                    for tb in range(4):
                        psT = psTp.tile([128, 1024], bf16, tag="psT")
                        for t in range(8):
                            tt = tb * 8 + t
                            nc.tensor.transpose(
                                psT[:, 128 * t:128 * t + 128],
                                xs[:, 128 * tt:128 * tt + 128],
                                ident[:],
                            )
                        nc.vector.tensor_copy(
                            atb[:, 1024 * tb:1024 * tb + 1024], psT[:])

                    # Each matmul gets its OWN psum bank and its own
                    # start/stop group: multiple groups (or one group with
                    # disjoint column writes) inside a single 2KB bank zero
                    # region fault the device when the bank is later read
                    # wholesale. A strided copy gathers 3 banks' outputs,
                    # touching only the written 64-column slice of each.
                    ob = obp.tile([128, N], bf16, tag="ob")
                    for g in range(22):          # groups of 3 j's (last has 1)
                        js = list(range(3 * g, min(3 * g + 3, NJ)))
                        psO = psOp.tile([128, 3, 512], f32, tag="psO")
                        for q, j in enumerate(js):
                            s, par = j // 2, j % 2
                            beta = par * 64
                            lhsT = atb[beta:beta + 64, 128 * s:128 * s + 128]
                            if (i % 2) == par:
                                rhs = wa[beta:beta + 64, i // 2,
                                         64 * j:64 * j + 64]
                            elif par == 0:
                                rhs = wb[0:64, (i - 1) // 2,
                                         64 * j:64 * j + 64]
                            else:
                                rhs = wb[64:128, i // 2, 64 * j:64 * j + 64]
                            nc.tensor.matmul(
                                psO[:, q, 0:64], lhsT, rhs,
                                start=True, stop=True, tile_position=(beta, 0))
                        ng = len(js)
                        dst = ob[:, 64 * js[0]:64 * js[0] + 64 * ng]
                        dst = dst.rearrange("p (q o) -> p q o", q=ng)
                        if g % 3 != 2:
                            nc.vector.tensor_copy(dst, psO[:, 0:ng, 0:64])
                        else:
                            nc.scalar.copy(dst, psO[:, 0:ng, 0:64])

                    # Two 2D stores on the two HWDGE rings.
                    nc.sync.dma_start(
                        o_d.ap()[2 * u, 64 * i:64 * i + 64, :], ob[0:64, :])
                    nc.scalar.dma_start(
                        o_d.ap()[2 * u + 1, 64 * i:64 * i + 64, :],
                        ob[64:128, :])

    nc.compile()
    return nc


def _get_nc():
    global _NC_CACHE
    if _NC_CACHE is None:
        _NC_CACHE = _build_nc()
    return _NC_CACHE


def kernel(x, weight):
    import ml_dtypes
    from concourse import bass_utils

    x = np.asarray(x, dtype=np.float32)
    w = np.asarray(weight, dtype=np.float32)
    assert x.shape == (B, M, K) and w.shape == (K, N)
    x16 = np.ascontiguousarray(x.astype(ml_dtypes.bfloat16))
    w16 = np.ascontiguousarray(w.astype(ml_dtypes.bfloat16))

    nc = _get_nc()
    in_maps = [
        {
            "x_shard": np.ascontiguousarray(x16[:, RPC * c:RPC * (c + 1), :]),
            "w_shard": np.ascontiguousarray(w16[RPC * c:RPC * (c + 1), :]),
        }
        for c in range(NCORES)
    ]
    res = bass_utils.run_bass_kernel_spmd(nc, in_maps,
                                          core_ids=list(range(NCORES)))
    out = np.concatenate(
        [res.results[c]["out_shard"] for c in range(NCORES)],
        axis=1).astype(np.float32)
    return out



# revision 5
# speedup vs baseline: 1.4863x; 1.4863x over previous
# Block-local matmul kernel for Trainium2 (8 NeuronCores, SPMD).
#
# Problem: out[b, i*64+r, j*64+o] = sum_c x[b, i*64+r, j*64+c] * W[i*64+c, j*64+o]
# with B=4, M=K=N=4096, 64x64 blocks. Embarrassingly parallel over (i, j).
#
# Sharding: block-row axis i across the 8 cores. Core p gets rows
# [512p, 512p+512) of x/out and rows [512p, 512p+512) of weight. No
# collectives; outputs are concatenated on the host.
#
# Per-core plan (memory-bound; HBM roofline ~200us for ~68MB):
#   - x loaded as 16 strips [128=(2 batches x 64 rows), 4096] with an
#     fp32->bf16 cast in the SWDGE DMA (SWDGE sprays 3D APs across all 16
#     SDMA engines; measured ~370GB/s in isolation).
#   - Stores are split into two 2D [64, 4096] DMAs on the two HWDGE rings
#     (nc.sync + nc.scalar). HWDGE assigns descriptors by the OUTER AP dim:
#     a 3D [2, 64, 4096] AP lands entirely on 2 SDMA engines (~54GB/s);
#     2D APs spray correctly.
#   - PE transposes each [128,128] bf16 tile into PSUM, giving A^T blocks
#     with partition = c (contraction), the matmul stationary.
#   - weight is cast to bf16 on the host; wa (natural strips) + wb
#     (partition-half swap) give W at both 64-partition alignments so rhs
#     always matches lhsT's base partition ((j%2)*64).
#   - Matmul: lhsT = A^T[c=64, (b2,r)=128] (stationary), rhs = W[c=64,o=64],
#     fp32 PSUM with NATURAL output layout -> contiguous stores.
#   - PSUM rule learned on HW: one matmul group per bank; readers may only
#     touch bytes the group wrote. Each matmul gets its own bank; a strided
#     copy gathers 3 banks, reading only the written 64-col slices.

import numpy as np

B = 4
M = K = N = 4096
NCORES = 8
RPC = M // NCORES  # 512 rows per core
NI = RPC // 64     # 8 i-blocks per core
NJ = N // 64       # 64 j-blocks

_NC_CACHE = None


def _build_nc():
    import concourse.tile as tile
    from concourse import bacc, masks, mybir

    f32 = mybir.dt.float32
    bf16 = mybir.dt.bfloat16

    nc = bacc.Bacc("TRN2", target_bir_lowering=False, debug=False,
                   num_devices=NCORES)
    x_d = nc.dram_tensor("x_shard", [B, RPC, K], bf16, kind="ExternalInput")
    w_d = nc.dram_tensor("w_shard", [RPC, N], bf16, kind="ExternalInput")
    o_d = nc.dram_tensor("out_shard", [B, RPC, N], bf16,
                         kind="ExternalOutput")

    with tile.TileContext(nc) as tc:
        with (
            tc.tile_pool(name="const", bufs=1) as constp,
            tc.tile_pool(name="wa", bufs=1) as wap,
            tc.tile_pool(name="wb", bufs=1) as wbp,
            tc.tile_pool(name="xs", bufs=3) as xp,
            tc.tile_pool(name="at", bufs=2) as atp,
            tc.tile_pool(name="ob", bufs=3) as obp,
            tc.tile_pool(name="psT", bufs=2, space="PSUM") as psTp,
            tc.tile_pool(name="psO", bufs=2, space="PSUM") as psOp,
        ):
            ident = constp.tile([128, 128], bf16)
            masks.make_identity(nc, ident[:])

            # wa[p, g, :] = weight row 128g + p. W(i) sits at partition base
            # (i%2)*64, strip g = i//2.
            wa = wap.tile([128, 4, N], bf16)
            nc.sync.dma_start(
                wa[:], w_d.ap().rearrange("(g p) n -> p g n", p=128))
            # wb = partition-half swap: odd i at base 0 (g=(i-1)//2), even i
            # at base 64 (g=i//2).
            wb = wbp.tile([128, 4, N], bf16)
            w_tpgn = w_d.ap().rearrange("(g t p) n -> t p g n", g=4, t=2, p=64)
            nc.scalar.dma_start(wb[0:64, :, :], w_tpgn[1])
            nc.scalar.dma_start(wb[64:128, :, :], w_tpgn[0])

            for u in range(2):        # batch pair (b in {2u, 2u+1})
                for i in range(NI):   # i-block within core
                    xs = xp.tile([128, K], bf16, tag="xs")
                    src = x_d.ap()[2 * u:2 * u + 2, 64 * i:64 * i + 64, :]
                    nc.gpsimd.dma_start(xs[:], src)

                    # Transpose all 32 [128,128] tiles; A^T(i, j) lands at
                    # partitions (j%2)*64 + c, free = (b2, r).
                    atb = atp.tile([128, 4096], bf16, tag="at")
                    for tb in range(4):
                        psT = psTp.tile([128, 1024], bf16, tag="psT")
                        for t in range(8):
                            tt = tb * 8 + t
                            nc.tensor.transpose(
                                psT[:, 128 * t:128 * t + 128],
                                xs[:, 128 * tt:128 * tt + 128],
                                ident[:],
                            )
                        nc.vector.tensor_copy(
                            atb[:, 1024 * tb:1024 * tb + 1024], psT[:])

                    # Each matmul gets its OWN psum bank and its own
                    # start/stop group: multiple groups (or one group with
                    # disjoint column writes) inside a single 2KB bank zero
                    # region fault the device when the bank is later read
                    # wholesale. A strided copy gathers 3 banks' outputs,
                    # touching only the written 64-column slice of each.
                    ob = obp.tile([128, N], bf16, tag="ob")
                    for g in range(22):          # groups of 3 j's (last has 1)
                        js = list(range(3 * g, min(3 * g + 3, NJ)))
                        psO = psOp.tile([128, 3, 512], f32, tag="psO")
                        for q, j in enumerate(js):
                            s, par = j // 2, j % 2
                            beta = par * 64
                            lhsT = atb[beta:beta + 64, 128 * s:128 * s + 128]
                            if (i % 2) == par:
                                rhs = wa[beta:beta + 64, i // 2,
                                         64 * j:64 * j + 64]
                            elif par == 0:
                                rhs = wb[0:64, (i - 1) // 2,
                                         64 * j:64 * j + 64]
                            else:
                                rhs = wb[64:128, i // 2, 64 * j:64 * j + 64]
                            nc.tensor.matmul(
                                psO[:, q, 0:64], lhsT, rhs,
                                start=True, stop=True, tile_position=(beta, 0))
                        ng = len(js)
                        dst = ob[:, 64 * js[0]:64 * js[0] + 64 * ng]
                        dst = dst.rearrange("p (q o) -> p q o", q=ng)
                        if g % 3 != 2:
                            nc.vector.tensor_copy(dst, psO[:, 0:ng, 0:64])
                        else:
                            nc.scalar.copy(dst, psO[:, 0:ng, 0:64])

                    # Two 2D stores on the two HWDGE rings.
                    nc.sync.dma_start(
                        o_d.ap()[2 * u, 64 * i:64 * i + 64, :], ob[0:64, :])
                    nc.scalar.dma_start(
                        o_d.ap()[2 * u + 1, 64 * i:64 * i + 64, :],
                        ob[64:128, :])

    nc.compile()
    return nc


def _get_nc():
    global _NC_CACHE
    if _NC_CACHE is None:
        _NC_CACHE = _build_nc()
    return _NC_CACHE


def kernel(x, weight):
    import ml_dtypes
    from concourse import bass_utils

    x = np.asarray(x, dtype=np.float32)
    w = np.asarray(weight, dtype=np.float32)
    assert x.shape == (B, M, K) and w.shape == (K, N)
    x16 = np.ascontiguousarray(x.astype(ml_dtypes.bfloat16))
    w16 = np.ascontiguousarray(w.astype(ml_dtypes.bfloat16))

    nc = _get_nc()
    in_maps = [
        {
            "x_shard": np.ascontiguousarray(x16[:, RPC * c:RPC * (c + 1), :]),
            "w_shard": np.ascontiguousarray(w16[RPC * c:RPC * (c + 1), :]),
        }
        for c in range(NCORES)
    ]
    res = bass_utils.run_bass_kernel_spmd(nc, in_maps,
                                          core_ids=list(range(NCORES)))
    out = np.concatenate(
        [res.results[c]["out_shard"] for c in range(NCORES)],
        axis=1).astype(np.float32)
    return out



# revision 15
# speedup vs baseline: 2.0158x; 1.3563x over previous
# Block-local matmul kernel for Trainium2 (8 NeuronCores, SPMD).
#
# Problem: out[b, i*64+r, j*64+o] = sum_c x[b, i*64+r, j*64+c] * W[i*64+c, j*64+o]
# with B=4, M=K=N=4096, 64x64 blocks. Embarrassingly parallel over (i, j).
#
# Sharding: block-row axis i across the 8 cores. Core p gets rows
# [512p, 512p+512) of x/out and rows [512p, 512p+512) of weight. No
# collectives; outputs are concatenated on the host.
#
# v3 design (HBM-bound; ~36MB/core on the wire):
#   - Host casts x/weight to bf16; output is stored bf16, upcast on the
#     host. Host also interleaves batch pairs into rows m = 2r + b2 so
#     each (u, i) strip is ONE [128, 4096] DRAM block.
#   - x strips load via dma_start_transpose (XBAR) on the HWDGE rings
#     into atb[p, s, m] = x'[u, i, m, 128s+p]. The transpose DEST MUST
#     BE CONTIGUOUS: a strided dest produces wrong output on HW (see
#     tile_matmul.py:408 comment; confirmed here as run-varying
#     corruption). The interleave makes the dest contiguous AND the
#     matmul lhsT a single free dim, and kills all PE transposes (v1
#     burned ~80us of PE on transpose+identity reloads; SWDGE x loads
#     ran at ~70GB/s due to 1KB packets - avoid SWDGE for bulk).
#   - W is materialized once in SBUF as block-diagonal pair tiles
#     wbd_i[p, s, o2] = blockdiag(W(i,2s), W(i,2s+1)); each j-pair is
#     one full-array K=128 matmul:
#       lhsT = atb[:, s, :] (stationary, [128, 128]),
#       rhs  = wbd_i[:, s, :],
#       out  = psum[m, o2] -> ob[:, 128s:128s+128] contiguous.
#     This halves LDWEIGHTS column count vs per-j k=64 matmuls (the v1
#     PE bottleneck: LdW streams M cols per matmul).
#   - PSUM rule learned on HW: one matmul group per 2KB bank; readers
#     may only touch bytes the group wrote. Each matmul gets its own
#     full bank ([128,512] f32 tile), evacuation reads only [:, 0:128].
#   - Evacuation copies alternate vector/scalar engines, casting
#     f32->bf16. Stores de-interleave batches with partition-stride-2
#     APs, one [64, 4096] bf16 store per ring per (u, i).

import contextlib

import numpy as np

B = 4
M = K = N = 4096
NCORES = 8
RPC = M // NCORES  # 512 rows per core
NI = RPC // 64     # 8 i-blocks per core
NS = 32            # j-pairs

_NC_CACHE = None


def _build_nc():
    import concourse.tile as tile
    from concourse import bacc, mybir

    f32 = mybir.dt.float32
    bf16 = mybir.dt.bfloat16

    nc = bacc.Bacc("TRN2", target_bir_lowering=False, debug=False,
                   num_devices=NCORES)
    # x_shard[u, i, m, k]: m = 2r + b2 interleaves batches 2u and 2u+1.
    x_d = nc.dram_tensor("x_shard", [2, NI, 128, K], bf16,
                         kind="ExternalInput")
    w_d = nc.dram_tensor("w_shard", [RPC, N], bf16, kind="ExternalInput")
    o_d = nc.dram_tensor("out_shard", [B, RPC, N], bf16,
                         kind="ExternalOutput")

    with tile.TileContext(nc) as tc, contextlib.ExitStack() as ex:
        wpools = [ex.enter_context(tc.tile_pool(name=f"wbd{i}", bufs=1))
                  for i in range(NI)]
        atp = ex.enter_context(tc.tile_pool(name="atb", bufs=3))
        obp = ex.enter_context(tc.tile_pool(name="ob", bufs=3))
        psp = ex.enter_context(tc.tile_pool(name="ps", bufs=2, space="PSUM"))

        # W block-diagonal pair tiles, one per i so the first matmuls
        # only wait on their own i's fill.
        # The XBAR transpose path is serialized by keeping ALL
        # dma_start_transpose on the sync ring (FIFO per ring);
        # concurrent transposes from both rings interleave 256B xbar
        # tiles and corrupt data. Everything else rides the scalar ring.
        wbds = []
        for i in range(NI):
            wbd = wpools[i].tile([128, NS, 128], bf16)
            nc.gpsimd.memset(wbd[0:64, :, 64:128], 0.0)
            nc.gpsimd.memset(wbd[64:128, :, 0:64], 0.0)
            w_src = w_d.ap()[64 * i:64 * i + 64, :].rearrange(
                "c (s o2) -> c s o2", o2=128)
            nc.scalar.dma_start(wbd[0:64, :, 0:64], w_src[:, :, 0:64])
            nc.scalar.dma_start(wbd[64:128, :, 64:128], w_src[:, :, 64:128])
            wbds.append(wbd)

        for u in range(2):        # batch pair (b in {2u, 2u+1})
            for i in range(NI):   # i-block within core
                atb = atp.tile([128, NS, 128], bf16, tag="atb")
                nc.sync.dma_start_transpose(atb[:], x_d.ap()[u, i])

                ob = obp.tile([128, N], bf16, tag="ob")
                for g in range(8):   # groups of 4 j-pairs -> 4 banks
                    # One matmul group per bank: q strides a full 2KB
                    # bank; the evacuation copy reads only the written
                    # 128-col f32 slice of each bank, casting to bf16.
                    ps = psp.tile([128, 4, 512], f32, tag="ps")
                    for q in range(4):
                        s = 4 * g + q
                        nc.tensor.matmul(
                            ps[:, q, 0:128], atb[:, s, :], wbds[i][:, s, :],
                            start=True, stop=True)
                    dst = ob[:, 512 * g:512 * g + 512].rearrange(
                        "p (q o) -> p q o", q=4)
                    if g % 2 == 0:
                        nc.vector.tensor_copy(dst, ps[:, 0:4, 0:128])
                    else:
                        nc.scalar.copy(dst, ps[:, 0:4, 0:128])

                # De-interleave on the HBM side: SBUF partition 2r+b2
                # -> batch 2u+b2, row 64i+r. SBUF src stays contiguous
                # (partition-strided SBUF APs don't lower correctly).
                dst = o_d.ap()[2 * u:2 * u + 2, 64 * i:64 * i + 64,
                               :].rearrange("b2 r n -> r b2 n")
                nc.scalar.dma_start(dst, ob[:])

    nc.compile()
    return nc


def _get_nc():
    global _NC_CACHE
    if _NC_CACHE is None:
        _NC_CACHE = _build_nc()
    return _NC_CACHE


def kernel(x, weight):
    import ml_dtypes
    from concourse import bass_utils

    x = np.asarray(x, dtype=np.float32)
    w = np.asarray(weight, dtype=np.float32)
    assert x.shape == (B, M, K) and w.shape == (K, N)
    x16 = x.astype(ml_dtypes.bfloat16)
    w16 = np.ascontiguousarray(w.astype(ml_dtypes.bfloat16))
    # Interleave rows: xi[u, ig, m=(r, b2), k] = x[2u+b2, 64ig+r, k].
    xi = np.ascontiguousarray(
        x16.reshape(2, 2, 64, 64, K).transpose(0, 2, 3, 1, 4)
    ).reshape(2, 64, 128, K)

    nc = _get_nc()
    in_maps = [
        {
            "x_shard": np.ascontiguousarray(xi[:, NI * c:NI * (c + 1)]),
            "w_shard": np.ascontiguousarray(w16[RPC * c:RPC * (c + 1), :]),
        }
        for c in range(NCORES)
    ]
    res = bass_utils.run_bass_kernel_spmd(nc, in_maps,
                                          core_ids=list(range(NCORES)))
    out = np.concatenate(
        [res.results[c]["out_shard"] for c in range(NCORES)],
        axis=1).astype(np.float32)
    return out
